# revision 41
# baseline (speedup 1.0000x reference)
"""MoE layer (top-2 of 8 experts, SwiGLU) on 8 Trainium2 NeuronCores.

Strategy (expert-parallel, sparse):
  - Host: router (logits -> softmax -> top-2, replicated from the reference
    semantics), gather each expert's tokens, pad to a uniform capacity C.
  - Device (SPMD, core c = expert c): y = (silu(xg @ Wg + bg) * (xg @ W1 + b1)) @ W2,
    scaled per-token by the combine weight. All matmuls in bf16 with fp32 PSUM
    accumulation. Inputs are sent pre-transposed so every matmul operand is a
    natural [K=128, *] slice.
  - Host: scatter-add the two expert contributions per token; + comb @ b2.

Self-contained: hardcodes shapes B=4, N=1024, D=1024, E=8, FF=4096, TOP_K=2.
"""

import os
import sys
import types

import numpy as np

for _p in ("/opt/trn_rl_repo",):
    if _p not in sys.path and os.path.isdir(_p):
        sys.path.insert(0, _p)

import ml_dtypes  # noqa: E402
import concourse.bass as bass  # noqa: E402
import concourse.mybir as mybir  # noqa: E402
import concourse.tile as tile  # noqa: E402
from concourse.bass_utils import run_bass_kernel_spmd  # noqa: E402


# ----------------------------------------------------------------------------
# NTFF profile hook shim: the container's stub `antenv` package lacks
# `axon_hooks`, so run_bass_kernel_spmd(trace=True) would silently skip
# profiling. Register the ctypes-based hook from trn_agent_boot ourselves.
def _install_ntff_hook():
    try:
        import antenv

        if "antenv.axon_hooks" in sys.modules:
            return
        m = types.ModuleType("antenv.axon_hooks")
        m._hook = None
        m.set_axon_ntff_profile_hook = lambda h: setattr(m, "_hook", h)
        m.get_axon_ntff_profile_hook = lambda: m._hook
        sys.modules["antenv.axon_hooks"] = m
        antenv.axon_hooks = m
        from trn_agent_boot.trn_boot import _ntff_profile_via_ctypes

        m.set_axon_ntff_profile_hook(
            _ntff_profile_via_ctypes("/opt/axon/libaxon_pjrt.so")
        )
    except Exception:
        pass


_install_ntff_hook()


# ----------------------------------------------------------------------------
# Post-pass: this container's walrus build rejects instructions carrying more
# than one sync wait ("Too many sync wait commands"). Move excess waits onto
# standalone NoOps inserted before the instruction on the same engine.
_WAITFIX_CTR = [0]


def _split_excess_waits(nc, max_waits: int = 1) -> int:
    n_split = 0
    for fn in nc.m.functions:
        for bb in fn.blocks:
            il = bb.instructions
            new = []
            changed = False
            for inst in il:
                si = inst.sync_info
                if si is not None and si.on_wait and len(si.on_wait) > max_waits:
                    waits = list(si.on_wait)
                    extra, keep = waits[:-max_waits], waits[-max_waits:]
                    for w in extra:
                        _WAITFIX_CTR[0] += 1
                        nop = mybir.InstNoOp(
                            name=f"wait-split-{_WAITFIX_CTR[0]}", ins=[], outs=[]
                        )
                        nop.engine = inst.engine
                        nop.sync_info = mybir.SyncInfo(on_wait=[w], on_update=[])
                        new.append(nop)
                        n_split += 1
                    inst.sync_info = mybir.SyncInfo(
                        on_wait=keep, on_update=list(si.on_update)
                    )
                    changed = True
                new.append(inst)
            if changed:
                il[:] = new
    return n_split


# ----------------------------------------------------------------------------
B, N, D, E, FF, TOP_K = 4, 1024, 1024, 8, 4096, 2
T = B * N
P = 128
KD = D // P  # 8 contraction chunks over D
KF = FF // P  # 32 contraction chunks over FF
NCORES = 8

F32 = mybir.dt.float32
AF = mybir.ActivationFunctionType

LAST_RESULT = None  # BassKernelResults of the most recent device dispatch


def _token_chunks(C):
    """Split [0, C) into near-equal chunks <= 512, multiples of 32.

    Equal-size chunks keep every matmul's free dim large (LDWEIGHTS stays
    hidden behind the stream) instead of 512+...+small-remainder."""
    n = -(-C // 512)
    base = (C // n) // 32 * 32
    rem = (C - base * n) // 32  # number of chunks that get +32
    sizes = [base + 32 if i < rem else base for i in range(n)]
    chunks = []
    off = 0
    for s in sizes:
        chunks.append((off, s))
        off += s
    assert off == C
    return chunks


def _build_program(C: int, dt, with_bias: bool):
    """One expert's fused SwiGLU MLP over C (padded) tokens.

    DRAM parameter layouts (host pre-arranges):
      xT : [KD, P, C]    xT[k, p, t] = x_gathered[t, k*128+p]
      w1 : [KF, P, KD*P] w1[f, p, k*128+c] = W1[k*128+p, f*128+c]
      wg : same as w1
      w2 : [KF, P, D]    w2[f, p, :] = W2[f*128+p, :]
      wc : [P, NTB]      wc[p, t] = combine_weight[t*128+p]
      b1f/bgf : [P, KF]  b1f[p, f] = b1[f*128+p]
      y  : [NTB, P, D]   y[t, p, :] = out[t*128+p, :]
    """
    NTB = -(-C // P)
    chunks = _token_chunks(C)
    # f=0 chunking: small leading chunk to shrink the kernel-head DMA wait
    if C >= 640:
        chunks0 = [(0, P)] + [(P + t0, tl) for (t0, tl) in _token_chunks(C - P)]
    else:
        chunks0 = chunks
    nc = bass.Bass()

    xT_d = nc.declare_dram_parameter("xT", [KD, P, C], dt, isOutput=False)
    w1_d = nc.declare_dram_parameter("w1", [KF, P, KD * P], dt, isOutput=False)
    wg_d = nc.declare_dram_parameter("wg", [KF, P, KD * P], dt, isOutput=False)
    w2_d = nc.declare_dram_parameter("w2", [KF, P, D], dt, isOutput=False)
    wc_d = nc.declare_dram_parameter("wc", [P, NTB], F32, isOutput=False)
    if with_bias:
        b1_d = nc.declare_dram_parameter("b1f", [P, KF], F32, isOutput=False)
        bg_d = nc.declare_dram_parameter("bgf", [P, KF], F32, isOutput=False)
    y_d = nc.declare_dram_parameter("y", [NTB, P, D], F32, isOutput=True)

    # Per-partition SBUF bytes if w2 is cached resident: xT + aT + w2 + streams
    sz = mybir.dt.size(dt)
    per_part = (KD * C + KF * C + KF * D) * sz + 31 * 1024
    cache_w2 = per_part <= 190 * 1024

    with tile.TileContext(nc) as tc:
        with tc.tile_pool(name="const", bufs=1) as constp:
            # Resident inputs go through the software DGE (gpsimd/Pool engine):
            # a separate DMA ring from the HWDGE one, so the w2/x preloads
            # don't queue behind (or throttle) the phase-A weight stream.
            xTt = constp.tile([P, KD, C], dt)
            # chunk-major emission: the first token chunk's pieces (needed by
            # the first accumulation group) lead the SWDGE ring
            for (t0, tl) in chunks0:
                for k in range(KD):
                    nc.gpsimd.dma_start(
                        out=xTt[:, k, t0 : t0 + tl], in_=xT_d[k, :, t0 : t0 + tl]
                    )
            wct = constp.tile([P, NTB], F32)
            nc.gpsimd.dma_start(out=wct[:], in_=wc_d[:])
            if with_bias:
                b1t = constp.tile([P, KF], F32)
                nc.gpsimd.dma_start(out=b1t[:], in_=b1_d[:])
                bgt = constp.tile([P, KF], F32)
                nc.gpsimd.dma_start(out=bgt[:], in_=bg_d[:])
            aT = constp.tile([P, KF, C], dt)
            w2t = constp.tile([P, KF, D], dt, name="w2t") if cache_w2 else None
            if cache_w2:
                for f in range(KF):
                    nc.gpsimd.dma_start(out=w2t[:, f, :], in_=w2_d[f])

            # ---- Phase A: h^T/g^T per 128-wide FF chunk; a = silu(g) * h ----
            with tc.tile_pool(name="wstream", bufs=4) as wp, tc.tile_pool(
                name="hgsb", bufs=4
            ) as hgp, tc.tile_pool(name="psA", bufs=3, space="PSUM") as psA:
                for f in range(KF):
                    w1f = wp.tile([P, KD, P], dt, tag="w1f")
                    wgf = wp.tile([P, KD, P], dt, tag="wgf")
                    if f == 0:
                        # first matmul's critical path: halves land on two queues
                        hk = KD // 2
                        nc.sync.dma_start(out=w1f[:, :hk, :], in_=w1_d[0, :, : hk * P])
                        nc.sync.dma_start(out=w1f[:, hk:, :], in_=w1_d[0, :, hk * P :])
                        nc.sync.dma_start(out=wgf[:, :hk, :], in_=wg_d[0, :, : hk * P])
                        nc.sync.dma_start(out=wgf[:, hk:, :], in_=wg_d[0, :, hk * P :])
                    else:
                        nc.sync.dma_start(out=w1f[:], in_=w1_d[f])
                        nc.sync.dma_start(out=wgf[:], in_=wg_d[f])
                    # f == 0 starts with a small 128-token chunk so the first
                    # matmuls wait on ~300KB of DMA instead of ~1.3MB
                    f_chunks = chunks0 if f == 0 else chunks
                    for (t0, tl) in f_chunks:
                        hps = psA.tile([P, 512], F32, tag="h")
                        gps = psA.tile([P, 512], F32, tag="g")
                        for k in range(KD):
                            nc.tensor.matmul(
                                hps[:, :tl],
                                w1f[:, k, :],
                                xTt[:, k, t0 : t0 + tl],
                                start=(k == 0),
                                stop=(k == KD - 1),
                            )
                        for k in range(KD):
                            nc.tensor.matmul(
                                gps[:, :tl],
                                wgf[:, k, :],
                                xTt[:, k, t0 : t0 + tl],
                                start=(k == 0),
                                stop=(k == KD - 1),
                            )
                        sg = hgp.tile([P, 512], F32, tag="sg")
                        if with_bias:
                            nc.scalar.activation(
                                sg[:, :tl], gps[:, :tl], AF.Silu, bias=bgt[:, f : f + 1]
                            )
                            hb = hgp.tile([P, 512], F32, tag="hb")
                            nc.scalar.activation(
                                hb[:, :tl],
                                hps[:, :tl],
                                AF.Identity,
                                bias=b1t[:, f : f + 1],
                            )
                            nc.vector.tensor_mul(
                                aT[:, f, t0 : t0 + tl], sg[:, :tl], hb[:, :tl]
                            )
                        else:
                            nc.scalar.activation(sg[:, :tl], gps[:, :tl], AF.Silu)
                            nc.vector.tensor_mul(
                                aT[:, f, t0 : t0 + tl], sg[:, :tl], hps[:, :tl]
                            )

            # ---- Phase B: y[t, :] = (a^T)^T @ W2, scaled by combine weight ----
            with tc.tile_pool(name="w2p", bufs=3) as w2p, tc.tile_pool(
                name="ysb", bufs=4
            ) as yp, tc.tile_pool(name="psB", bufs=1, space="PSUM") as psB:
                GROUP = 4  # 4 token chunks x 2 halves of D = 8 PSUM banks
                starts = list(range(0, NTB, GROUP))
                groups = [
                    list(range(g0, min(g0 + GROUP, NTB))) for g0 in starts
                ]
                if len(groups[-1]) > 1:
                    # end on a 1-chunk group: the after-last-matmul eviction
                    # tail is 2 tiles instead of 8
                    groups = groups[:-1] + [groups[-1][:-1], groups[-1][-1:]]
                for ts in groups:
                    g0 = ts[0]
                    ytiles = {}
                    for t in ts:
                        for dn in range(2):
                            ytiles[(t, dn)] = psB.tile(
                                [P, 512],
                                F32,
                                tag=f"y{t - g0}_{dn}",
                                name=f"ypsum_{t - g0}_{dn}",
                            )
                    for f in range(KF):
                        if cache_w2:
                            w2f = w2t[:, f, :]
                        else:
                            w2f = w2p.tile([P, D], dt, tag="w2f")
                            nc.sync.dma_start(out=w2f[:], in_=w2_d[f])
                        for t in ts:
                            mt = min(P, C - t * P)
                            for dn in range(2):
                                nc.tensor.matmul(
                                    ytiles[(t, dn)][:mt],
                                    aT[:, f, t * P : t * P + mt],
                                    w2f[:, dn * 512 : (dn + 1) * 512],
                                    start=(f == 0),
                                    stop=(f == KF - 1),
                                )
                    for t in ts:
                        mt = min(P, C - t * P)
                        for dn in range(2):
                            ysb = yp.tile([P, 512], F32, tag="ysb")
                            if (t + dn) % 2 == 0:
                                nc.scalar.activation(
                                    ysb[:mt],
                                    ytiles[(t, dn)][:mt],
                                    AF.Copy,
                                    scale=wct[:mt, t : t + 1],
                                )
                            else:
                                nc.vector.tensor_scalar_mul(
                                    ysb[:mt], ytiles[(t, dn)][:mt], wct[:mt, t : t + 1]
                                )
                            nc.sync.dma_start(
                                out=y_d[t, :mt, dn * 512 : (dn + 1) * 512],
                                in_=ysb[:mt],
                            )

    _split_excess_waits(nc)
    return nc


_PROG_CACHE = {}


def _get_program(C, dt_name, with_bias):
    key = (C, dt_name, with_bias)
    if key not in _PROG_CACHE:
        dt = {"bf16": mybir.dt.bfloat16, "f32r": mybir.dt.float32r, "f32": F32}[
            dt_name
        ]
        _PROG_CACHE[key] = _build_program(C, dt, with_bias)
    return _PROG_CACHE[key]


# ----------------------------------------------------------------------------
def _route(xf, Wr):
    """Replicate the reference router bit-for-bit (jax on CPU), falling back
    to float64 numpy for the picks if jax is unavailable.

    Returns probs [T,E] f32, topi [T,K] int, topv_norm [T,K] f32."""
    try:
        import jax
        import jax.numpy as jnp

        cpu = jax.devices("cpu")[0]
        with jax.default_device(cpu):
            logits = jnp.asarray(xf) @ jnp.asarray(Wr).T
            probs = jax.nn.softmax(logits, axis=-1)
            topv, topi = jax.lax.top_k(probs, TOP_K)
            probs = np.asarray(probs)
            topv = np.asarray(topv)
            topi = np.asarray(topi)
    except Exception:
        l64 = xf.astype(np.float64) @ Wr.astype(np.float64).T
        logits = (xf @ Wr.T).astype(np.float32)
        m = logits.max(-1, keepdims=True)
        e = np.exp(logits - m)
        probs = e / e.sum(-1, keepdims=True)
        topi = np.argsort(-l64, axis=-1, kind="stable")[:, :TOP_K]
        topv = np.take_along_axis(probs, topi, -1)
    topv = topv / (topv.sum(-1, keepdims=True) + 1e-6)
    return probs.astype(np.float32), topi, topv.astype(np.float32)


def _prep_weights(W1, Wg, W2, np_dt):
    """Host re-layouts (cached on id) so device DMAs are partition-contiguous."""
    key = (id(W1), id(Wg), id(W2), np_dt)
    cached = _prep_weights._cache.get(key)
    if cached is not None:
        return cached
    # w1[e][f, p, k*128+c] = W1[e, k*128+p, f*128+c]
    w1 = np.ascontiguousarray(
        W1.reshape(E, KD, P, KF, P).transpose(0, 3, 2, 1, 4).reshape(E, KF, P, KD * P)
    ).astype(np_dt)
    wg = np.ascontiguousarray(
        Wg.reshape(E, KD, P, KF, P).transpose(0, 3, 2, 1, 4).reshape(E, KF, P, KD * P)
    ).astype(np_dt)
    w2 = np.ascontiguousarray(W2.reshape(E, KF, P, D)).astype(np_dt)
    _prep_weights._cache = {key: (w1, wg, w2)}
    return w1, wg, w2


_prep_weights._cache = {}


def _expert_mlp_host(xg, W1e, b1e, Wge, bge, W2e, b2e):
    """fp32 numpy expert forward for the (pathological) host fallback."""
    h = xg @ W1e + b1e
    g = xg @ Wge + bge
    sg = g / (1.0 + np.exp(-g))
    return (sg * h) @ W2e + b2e


def kernel(x, Wr, W1, b1, Wg, bg, W2, b2):
    global LAST_RESULT
    dt_name = "bf16"
    np_dt = ml_dtypes.bfloat16

    x = np.asarray(x, np.float32)
    Wr = np.asarray(Wr, np.float32)
    W1 = np.asarray(W1, np.float32)
    Wg = np.asarray(Wg, np.float32)
    W2 = np.asarray(W2, np.float32)
    b1 = np.asarray(b1, np.float32)
    bg = np.asarray(bg, np.float32)
    b2 = np.asarray(b2, np.float32)

    xf = x.reshape(T, D)
    probs, topi, topv = _route(xf, Wr)

    # combine weights comb[t, e]
    comb = np.zeros((T, E), np.float32)
    np.add.at(comb, (np.arange(T)[:, None], topi), topv)

    # tokens per expert (ascending token order, matching reference accumulation);
    # membership from topi, not comb != 0, so a 0-weight selection still routes
    sel = np.zeros((T, E), bool)
    sel[np.arange(T)[:, None], topi] = True
    idx = [np.nonzero(sel[:, e])[0] for e in range(E)]
    counts = np.array([len(i) for i in idx])

    # Device capacity: padding costs PE time per token (phase A) and per
    # 128-token PSUM chunk (phase B), so cap at a clean 1024 and compute the
    # few overflow (token, expert) pairs on the host in exact fp32.
    C = max(512, int(np.ceil(counts.max() / 32)) * 32)
    if counts.max() > 1024:
        C = 1024
    NTB = -(-C // P)

    if C > 1792:
        # Pathologically imbalanced routing: capacity would overflow SBUF.
        # Never triggers for a near-uniform router; correctness fallback only.
        out = np.zeros((T, D), np.float32)
        for e in range(E):
            ic = idx[e]
            if len(ic):
                out[ic] += comb[ic, e][:, None] * _expert_mlp_host(
                    xf[ic], W1[e], b1[e], Wg[e], bg[e], W2[e], b2[e]
                )
        countsf = np.zeros((E,), np.float32)
        np.add.at(countsf, topi.reshape(-1), np.float32(1.0))
        frac = countsf / (countsf.sum() + np.float32(1e-6))
        lb = np.float32((frac * probs.sum(0)).sum() * E)
        return out.reshape(B, N, D), lb

    with_bias = bool(b1.any() or bg.any())
    nc = _get_program(C, dt_name, with_bias)

    w1h, wgh, w2h = _prep_weights(W1, Wg, W2, np_dt)

    in_maps = []
    for c in range(NCORES):
        ic = idx[c][:C]
        ncnt = len(ic)
        xg = np.zeros((C, D), np.float32)
        xg[:ncnt] = xf[ic]
        xT = np.ascontiguousarray(xg.T.reshape(KD, P, C)).astype(np_dt)
        wc = np.zeros((NTB * P,), np.float32)
        wc[:ncnt] = comb[ic, c]
        m = {
            "xT": xT,
            "w1": w1h[c],
            "wg": wgh[c],
            "w2": w2h[c],
            "wc": np.ascontiguousarray(wc.reshape(NTB, P).T),
        }
        if with_bias:
            m["b1f"] = np.ascontiguousarray(b1[c].reshape(KF, P).T)
            m["bgf"] = np.ascontiguousarray(bg[c].reshape(KF, P).T)
        in_maps.append(m)

    res = run_bass_kernel_spmd(nc, in_maps, list(range(NCORES)))
    LAST_RESULT = res

    out = np.zeros((T, D), np.float32)
    for c in range(NCORES):
        ic = idx[c][:C]
        yc = np.asarray(res.results[c]["y"], np.float32).reshape(NTB * P, D)
        out[ic] += yc[: len(ic)]
        ov = idx[c][C:]
        if len(ov):
            # b2 excluded here: the global `comb @ b2` term below already
            # covers every selected (token, expert) pair
            out[ov] += comb[ov, c][:, None] * _expert_mlp_host(
                xf[ov], W1[c], b1[c], Wg[c], bg[c], W2[c], np.float32(0.0)
            )

    if b2.any():
        out += comb @ b2

    # load-balance aux loss, replicated from the reference
    countsf = np.zeros((E,), np.float32)
    np.add.at(countsf, topi.reshape(-1), np.float32(1.0))
    frac = countsf / (countsf.sum() + np.float32(1e-6))
    lb = np.float32((frac * probs.sum(0)).sum() * E)

    return out.reshape(B, N, D), lb


# revision 42
# speedup vs baseline: 1.0088x; 1.0088x over previous
"""MoE layer (top-2 of 8 experts, SwiGLU) on 8 Trainium2 NeuronCores.

Strategy (expert-parallel, sparse):
  - Host: router (logits -> softmax -> top-2, replicated from the reference
    semantics), gather each expert's tokens, pad to a uniform capacity C.
  - Device (SPMD, core c = expert c): y = (silu(xg @ Wg + bg) * (xg @ W1 + b1)) @ W2,
    scaled per-token by the combine weight. All matmuls in bf16 with fp32 PSUM
    accumulation. Inputs are sent pre-transposed so every matmul operand is a
    natural [K=128, *] slice.
  - Host: scatter-add the two expert contributions per token; + comb @ b2.

Self-contained: hardcodes shapes B=4, N=1024, D=1024, E=8, FF=4096, TOP_K=2.
"""

import os
import sys
import types

import numpy as np

for _p in ("/opt/trn_rl_repo",):
    if _p not in sys.path and os.path.isdir(_p):
        sys.path.insert(0, _p)

import ml_dtypes  # noqa: E402
import concourse.bass as bass  # noqa: E402
import concourse.mybir as mybir  # noqa: E402
import concourse.tile as tile  # noqa: E402
from concourse.bass_utils import run_bass_kernel_spmd  # noqa: E402


# ----------------------------------------------------------------------------
# NTFF profile hook shim: the container's stub `antenv` package lacks
# `axon_hooks`, so run_bass_kernel_spmd(trace=True) would silently skip
# profiling. Register the ctypes-based hook from trn_agent_boot ourselves.
def _install_ntff_hook():
    try:
        import antenv

        if "antenv.axon_hooks" in sys.modules:
            return
        m = types.ModuleType("antenv.axon_hooks")
        m._hook = None
        m.set_axon_ntff_profile_hook = lambda h: setattr(m, "_hook", h)
        m.get_axon_ntff_profile_hook = lambda: m._hook
        sys.modules["antenv.axon_hooks"] = m
        antenv.axon_hooks = m
        from trn_agent_boot.trn_boot import _ntff_profile_via_ctypes

        m.set_axon_ntff_profile_hook(
            _ntff_profile_via_ctypes("/opt/axon/libaxon_pjrt.so")
        )
    except Exception:
        pass


_install_ntff_hook()


# ----------------------------------------------------------------------------
# Post-pass: this container's walrus build rejects instructions carrying more
# than one sync wait ("Too many sync wait commands"). Move excess waits onto
# standalone NoOps inserted before the instruction on the same engine.
_WAITFIX_CTR = [0]


def _split_excess_waits(nc, max_waits: int = 1) -> int:
    n_split = 0
    for fn in nc.m.functions:
        for bb in fn.blocks:
            il = bb.instructions
            new = []
            changed = False
            for inst in il:
                si = inst.sync_info
                if si is not None and si.on_wait and len(si.on_wait) > max_waits:
                    waits = list(si.on_wait)
                    extra, keep = waits[:-max_waits], waits[-max_waits:]
                    for w in extra:
                        _WAITFIX_CTR[0] += 1
                        nop = mybir.InstNoOp(
                            name=f"wait-split-{_WAITFIX_CTR[0]}", ins=[], outs=[]
                        )
                        nop.engine = inst.engine
                        nop.sync_info = mybir.SyncInfo(on_wait=[w], on_update=[])
                        new.append(nop)
                        n_split += 1
                    inst.sync_info = mybir.SyncInfo(
                        on_wait=keep, on_update=list(si.on_update)
                    )
                    changed = True
                new.append(inst)
            if changed:
                il[:] = new
    return n_split


# ----------------------------------------------------------------------------
B, N, D, E, FF, TOP_K = 4, 1024, 1024, 8, 4096, 2
T = B * N
P = 128
KD = D // P  # 8 contraction chunks over D
KF = FF // P  # 32 contraction chunks over FF
NCORES = 8

F32 = mybir.dt.float32
AF = mybir.ActivationFunctionType

LAST_RESULT = None  # BassKernelResults of the most recent device dispatch


def _token_chunks(C):
    """Split [0, C) into near-equal chunks <= 512, multiples of 32.

    Equal-size chunks keep every matmul's free dim large (LDWEIGHTS stays
    hidden behind the stream) instead of 512+...+small-remainder."""
    n = -(-C // 512)
    base = (C // n) // 32 * 32
    rem = (C - base * n) // 32  # number of chunks that get +32
    sizes = [base + 32 if i < rem else base for i in range(n)]
    chunks = []
    off = 0
    for s in sizes:
        chunks.append((off, s))
        off += s
    assert off == C
    return chunks


def _build_program(C: int, dt, with_bias: bool):
    """One expert's fused SwiGLU MLP over C (padded) tokens.

    DRAM parameter layouts (host pre-arranges):
      xT : [KD, P, C]    xT[k, p, t] = x_gathered[t, k*128+p]
      w1 : [KF, P, KD*P] w1[f, p, k*128+c] = W1[k*128+p, f*128+c]
      wg : same as w1
      w2 : [KF, P, D]    w2[f, p, :] = W2[f*128+p, :]
      wc : [P, NTB]      wc[p, t] = combine_weight[t*128+p]
      b1f/bgf : [P, KF]  b1f[p, f] = b1[f*128+p]
      y  : [NTB, P, D]   y[t, p, :] = out[t*128+p, :]
    """
    NTB = -(-C // P)
    chunks = _token_chunks(C)
    # f=0 chunking: small leading chunk to shrink the kernel-head DMA wait
    if C >= 640:
        chunks0 = [(0, P)] + [(P + t0, tl) for (t0, tl) in _token_chunks(C - P)]
    else:
        chunks0 = chunks
    nc = bass.Bass()

    xT_d = nc.declare_dram_parameter("xT", [KD, P, C], dt, isOutput=False)
    w1_d = nc.declare_dram_parameter("w1", [KF, P, KD * P], dt, isOutput=False)
    wg_d = nc.declare_dram_parameter("wg", [KF, P, KD * P], dt, isOutput=False)
    w2_d = nc.declare_dram_parameter("w2", [KF, P, D], dt, isOutput=False)
    wc_d = nc.declare_dram_parameter("wc", [P, NTB], F32, isOutput=False)
    if with_bias:
        b1_d = nc.declare_dram_parameter("b1f", [P, KF], F32, isOutput=False)
        bg_d = nc.declare_dram_parameter("bgf", [P, KF], F32, isOutput=False)
    y_d = nc.declare_dram_parameter("y", [NTB, P, D], F32, isOutput=True)

    # Per-partition SBUF bytes if w2 is cached resident: xT + aT + w2 + streams
    sz = mybir.dt.size(dt)
    per_part = (KD * C + KF * C + KF * D) * sz + 31 * 1024
    cache_w2 = per_part <= 190 * 1024

    with tile.TileContext(nc) as tc:
        with tc.tile_pool(name="const", bufs=1) as constp:
            # Resident inputs go through the software DGE (gpsimd/Pool engine):
            # a separate DMA ring from the HWDGE one, so the w2/x preloads
            # don't queue behind (or throttle) the phase-A weight stream.
            xTt = constp.tile([P, KD, C], dt)
            # chunk-major emission: the first token chunk's pieces (needed by
            # the first accumulation group) lead the SWDGE ring
            for (t0, tl) in chunks0:
                for k in range(KD):
                    nc.gpsimd.dma_start(
                        out=xTt[:, k, t0 : t0 + tl], in_=xT_d[k, :, t0 : t0 + tl]
                    )
            wct = constp.tile([P, NTB], F32)
            nc.gpsimd.dma_start(out=wct[:], in_=wc_d[:])
            if with_bias:
                b1t = constp.tile([P, KF], F32)
                nc.gpsimd.dma_start(out=b1t[:], in_=b1_d[:])
                bgt = constp.tile([P, KF], F32)
                nc.gpsimd.dma_start(out=bgt[:], in_=bg_d[:])
            aT = constp.tile([P, KF, C], dt)
            w2t = constp.tile([P, KF, D], dt, name="w2t") if cache_w2 else None
            if cache_w2:
                for f in range(KF):
                    nc.gpsimd.dma_start(out=w2t[:, f, :], in_=w2_d[f])

            # ---- Phase A: h^T/g^T per 128-wide FF chunk; a = silu(g) * h ----
            with tc.tile_pool(name="wstream", bufs=4) as wp, tc.tile_pool(
                name="hgsb", bufs=4
            ) as hgp, tc.tile_pool(name="psA", bufs=3, space="PSUM") as psA:
                for f in range(KF):
                    w1f = wp.tile([P, KD, P], dt, tag="w1f")
                    wgf = wp.tile([P, KD, P], dt, tag="wgf")
                    if f == 0:
                        # first matmul's critical path: halves land on two queues
                        hk = KD // 2
                        nc.sync.dma_start(out=w1f[:, :hk, :], in_=w1_d[0, :, : hk * P])
                        nc.sync.dma_start(out=w1f[:, hk:, :], in_=w1_d[0, :, hk * P :])
                        nc.sync.dma_start(out=wgf[:, :hk, :], in_=wg_d[0, :, : hk * P])
                        nc.sync.dma_start(out=wgf[:, hk:, :], in_=wg_d[0, :, hk * P :])
                    else:
                        nc.sync.dma_start(out=w1f[:], in_=w1_d[f])
                        nc.sync.dma_start(out=wgf[:], in_=wg_d[f])
                    # f == 0 starts with a small 128-token chunk so the first
                    # matmuls wait on ~300KB of DMA instead of ~1.3MB
                    f_chunks = chunks0 if f == 0 else chunks
                    for (t0, tl) in f_chunks:
                        hps = psA.tile([P, 512], F32, tag="h")
                        gps = psA.tile([P, 512], F32, tag="g")
                        for k in range(KD):
                            nc.tensor.matmul(
                                hps[:, :tl],
                                w1f[:, k, :],
                                xTt[:, k, t0 : t0 + tl],
                                start=(k == 0),
                                stop=(k == KD - 1),
                            )
                        for k in range(KD):
                            nc.tensor.matmul(
                                gps[:, :tl],
                                wgf[:, k, :],
                                xTt[:, k, t0 : t0 + tl],
                                start=(k == 0),
                                stop=(k == KD - 1),
                            )
                        sg = hgp.tile([P, 512], F32, tag="sg")
                        if with_bias:
                            nc.scalar.activation(
                                sg[:, :tl], gps[:, :tl], AF.Silu, bias=bgt[:, f : f + 1]
                            )
                            hb = hgp.tile([P, 512], F32, tag="hb")
                            nc.scalar.activation(
                                hb[:, :tl],
                                hps[:, :tl],
                                AF.Identity,
                                bias=b1t[:, f : f + 1],
                            )
                            nc.vector.tensor_mul(
                                aT[:, f, t0 : t0 + tl], sg[:, :tl], hb[:, :tl]
                            )
                        else:
                            nc.scalar.activation(sg[:, :tl], gps[:, :tl], AF.Silu)
                            nc.vector.tensor_mul(
                                aT[:, f, t0 : t0 + tl], sg[:, :tl], hps[:, :tl]
                            )

            # ---- Phase B: y[t, :] = (a^T)^T @ W2, scaled by combine weight ----
            with tc.tile_pool(name="w2p", bufs=3) as w2p, tc.tile_pool(
                name="ysb", bufs=4
            ) as yp, tc.tile_pool(name="psB", bufs=1, space="PSUM") as psB:
                GROUP = 4  # 4 token chunks x 2 halves of D = 8 PSUM banks
                starts = list(range(0, NTB, GROUP))
                groups = [
                    list(range(g0, min(g0 + GROUP, NTB))) for g0 in starts
                ]
                if len(groups[-1]) > 1:
                    # end on a 1-chunk group: the after-last-matmul eviction
                    # tail is 2 tiles instead of 8
                    groups = groups[:-1] + [groups[-1][:-1], groups[-1][-1:]]
                for ts in groups:
                    g0 = ts[0]
                    ytiles = {}
                    for t in ts:
                        for dn in range(2):
                            ytiles[(t, dn)] = psB.tile(
                                [P, 512],
                                F32,
                                tag=f"y{t - g0}_{dn}",
                                name=f"ypsum_{t - g0}_{dn}",
                            )
                    if cache_w2:
                        # K-contiguous: all 32 f-accumulations back-to-back
                        # into one PSUM bank before moving to the next — the
                        # first chains run while phase A's tail still drains
                        # its banks, and the PE never cycles banks mid-chain.
                        for t in ts:
                            mt = min(P, C - t * P)
                            for dn in range(2):
                                for f in range(KF):
                                    nc.tensor.matmul(
                                        ytiles[(t, dn)][:mt],
                                        aT[:, f, t * P : t * P + mt],
                                        w2t[:, f, dn * 512 : (dn + 1) * 512],
                                        start=(f == 0),
                                        stop=(f == KF - 1),
                                    )
                    else:
                        for f in range(KF):
                            w2f = w2p.tile([P, D], dt, tag="w2f")
                            nc.sync.dma_start(out=w2f[:], in_=w2_d[f])
                            for t in ts:
                                mt = min(P, C - t * P)
                                for dn in range(2):
                                    nc.tensor.matmul(
                                        ytiles[(t, dn)][:mt],
                                        aT[:, f, t * P : t * P + mt],
                                        w2f[:, dn * 512 : (dn + 1) * 512],
                                        start=(f == 0),
                                        stop=(f == KF - 1),
                                    )
                    for t in ts:
                        mt = min(P, C - t * P)
                        for dn in range(2):
                            ysb = yp.tile([P, 512], F32, tag="ysb")
                            if (t + dn) % 2 == 0:
                                nc.scalar.activation(
                                    ysb[:mt],
                                    ytiles[(t, dn)][:mt],
                                    AF.Copy,
                                    scale=wct[:mt, t : t + 1],
                                )
                            else:
                                nc.vector.tensor_scalar_mul(
                                    ysb[:mt], ytiles[(t, dn)][:mt], wct[:mt, t : t + 1]
                                )
                            nc.sync.dma_start(
                                out=y_d[t, :mt, dn * 512 : (dn + 1) * 512],
                                in_=ysb[:mt],
                            )

    _split_excess_waits(nc)
    return nc


_PROG_CACHE = {}


def _get_program(C, dt_name, with_bias):
    key = (C, dt_name, with_bias)
    if key not in _PROG_CACHE:
        dt = {"bf16": mybir.dt.bfloat16, "f32r": mybir.dt.float32r, "f32": F32}[
            dt_name
        ]
        _PROG_CACHE[key] = _build_program(C, dt, with_bias)
    return _PROG_CACHE[key]


# ----------------------------------------------------------------------------
def _route(xf, Wr):
    """Replicate the reference router bit-for-bit (jax on CPU), falling back
    to float64 numpy for the picks if jax is unavailable.

    Returns probs [T,E] f32, topi [T,K] int, topv_norm [T,K] f32."""
    try:
        import jax
        import jax.numpy as jnp

        cpu = jax.devices("cpu")[0]
        with jax.default_device(cpu):
            logits = jnp.asarray(xf) @ jnp.asarray(Wr).T
            probs = jax.nn.softmax(logits, axis=-1)
            topv, topi = jax.lax.top_k(probs, TOP_K)
            probs = np.asarray(probs)
            topv = np.asarray(topv)
            topi = np.asarray(topi)
    except Exception:
        l64 = xf.astype(np.float64) @ Wr.astype(np.float64).T
        logits = (xf @ Wr.T).astype(np.float32)
        m = logits.max(-1, keepdims=True)
        e = np.exp(logits - m)
        probs = e / e.sum(-1, keepdims=True)
        topi = np.argsort(-l64, axis=-1, kind="stable")[:, :TOP_K]
        topv = np.take_along_axis(probs, topi, -1)
    topv = topv / (topv.sum(-1, keepdims=True) + 1e-6)
    return probs.astype(np.float32), topi, topv.astype(np.float32)


def _prep_weights(W1, Wg, W2, np_dt):
    """Host re-layouts (cached on id) so device DMAs are partition-contiguous."""
    key = (id(W1), id(Wg), id(W2), np_dt)
    cached = _prep_weights._cache.get(key)
    if cached is not None:
        return cached
    # w1[e][f, p, k*128+c] = W1[e, k*128+p, f*128+c]
    w1 = np.ascontiguousarray(
        W1.reshape(E, KD, P, KF, P).transpose(0, 3, 2, 1, 4).reshape(E, KF, P, KD * P)
    ).astype(np_dt)
    wg = np.ascontiguousarray(
        Wg.reshape(E, KD, P, KF, P).transpose(0, 3, 2, 1, 4).reshape(E, KF, P, KD * P)
    ).astype(np_dt)
    w2 = np.ascontiguousarray(W2.reshape(E, KF, P, D)).astype(np_dt)
    _prep_weights._cache = {key: (w1, wg, w2)}
    return w1, wg, w2


_prep_weights._cache = {}


def _expert_mlp_host(xg, W1e, b1e, Wge, bge, W2e, b2e):
    """fp32 numpy expert forward for the (pathological) host fallback."""
    h = xg @ W1e + b1e
    g = xg @ Wge + bge
    sg = g / (1.0 + np.exp(-g))
    return (sg * h) @ W2e + b2e


def kernel(x, Wr, W1, b1, Wg, bg, W2, b2):
    global LAST_RESULT
    dt_name = "bf16"
    np_dt = ml_dtypes.bfloat16

    x = np.asarray(x, np.float32)
    Wr = np.asarray(Wr, np.float32)
    W1 = np.asarray(W1, np.float32)
    Wg = np.asarray(Wg, np.float32)
    W2 = np.asarray(W2, np.float32)
    b1 = np.asarray(b1, np.float32)
    bg = np.asarray(bg, np.float32)
    b2 = np.asarray(b2, np.float32)

    xf = x.reshape(T, D)
    probs, topi, topv = _route(xf, Wr)

    # combine weights comb[t, e]
    comb = np.zeros((T, E), np.float32)
    np.add.at(comb, (np.arange(T)[:, None], topi), topv)

    # tokens per expert (ascending token order, matching reference accumulation);
    # membership from topi, not comb != 0, so a 0-weight selection still routes
    sel = np.zeros((T, E), bool)
    sel[np.arange(T)[:, None], topi] = True
    idx = [np.nonzero(sel[:, e])[0] for e in range(E)]
    counts = np.array([len(i) for i in idx])

    # Device capacity: padding costs PE time per token (phase A) and per
    # 128-token PSUM chunk (phase B), so cap at a clean 1024 and compute the
    # few overflow (token, expert) pairs on the host in exact fp32.
    C = max(512, int(np.ceil(counts.max() / 32)) * 32)
    if counts.max() > 1024:
        C = 1024
    NTB = -(-C // P)

    if C > 1792:
        # Pathologically imbalanced routing: capacity would overflow SBUF.
        # Never triggers for a near-uniform router; correctness fallback only.
        out = np.zeros((T, D), np.float32)
        for e in range(E):
            ic = idx[e]
            if len(ic):
                out[ic] += comb[ic, e][:, None] * _expert_mlp_host(
                    xf[ic], W1[e], b1[e], Wg[e], bg[e], W2[e], b2[e]
                )
        countsf = np.zeros((E,), np.float32)
        np.add.at(countsf, topi.reshape(-1), np.float32(1.0))
        frac = countsf / (countsf.sum() + np.float32(1e-6))
        lb = np.float32((frac * probs.sum(0)).sum() * E)
        return out.reshape(B, N, D), lb

    with_bias = bool(b1.any() or bg.any())
    nc = _get_program(C, dt_name, with_bias)

    w1h, wgh, w2h = _prep_weights(W1, Wg, W2, np_dt)

    in_maps = []
    for c in range(NCORES):
        ic = idx[c][:C]
        ncnt = len(ic)
        xg = np.zeros((C, D), np.float32)
        xg[:ncnt] = xf[ic]
        xT = np.ascontiguousarray(xg.T.reshape(KD, P, C)).astype(np_dt)
        wc = np.zeros((NTB * P,), np.float32)
        wc[:ncnt] = comb[ic, c]
        m = {
            "xT": xT,
            "w1": w1h[c],
            "wg": wgh[c],
            "w2": w2h[c],
            "wc": np.ascontiguousarray(wc.reshape(NTB, P).T),
        }
        if with_bias:
            m["b1f"] = np.ascontiguousarray(b1[c].reshape(KF, P).T)
            m["bgf"] = np.ascontiguousarray(bg[c].reshape(KF, P).T)
        in_maps.append(m)

    res = run_bass_kernel_spmd(nc, in_maps, list(range(NCORES)))
    LAST_RESULT = res

    out = np.zeros((T, D), np.float32)
    for c in range(NCORES):
        ic = idx[c][:C]
        yc = np.asarray(res.results[c]["y"], np.float32).reshape(NTB * P, D)
        out[ic] += yc[: len(ic)]
        ov = idx[c][C:]
        if len(ov):
            # b2 excluded here: the global `comb @ b2` term below already
            # covers every selected (token, expert) pair
            out[ov] += comb[ov, c][:, None] * _expert_mlp_host(
                xf[ov], W1[c], b1[c], Wg[c], bg[c], W2[c], np.float32(0.0)
            )

    if b2.any():
        out += comb @ b2

    # load-balance aux loss, replicated from the reference
    countsf = np.zeros((E,), np.float32)
    np.add.at(countsf, topi.reshape(-1), np.float32(1.0))
    frac = countsf / (countsf.sum() + np.float32(1e-6))
    lb = np.float32((frac * probs.sum(0)).sum() * E)

    return out.reshape(B, N, D), lb


# revision 43
# speedup vs baseline: 1.0100x; 1.0012x over previous
"""MoE layer (top-2 of 8 experts, SwiGLU) on 8 Trainium2 NeuronCores.

Strategy (expert-parallel, sparse):
  - Host: router (logits -> softmax -> top-2, replicated from the reference
    semantics), gather each expert's tokens, pad to a uniform capacity C.
  - Device (SPMD, core c = expert c): y = (silu(xg @ Wg + bg) * (xg @ W1 + b1)) @ W2,
    scaled per-token by the combine weight. All matmuls in bf16 with fp32 PSUM
    accumulation. Inputs are sent pre-transposed so every matmul operand is a
    natural [K=128, *] slice.
  - Host: scatter-add the two expert contributions per token; + comb @ b2.

Self-contained: hardcodes shapes B=4, N=1024, D=1024, E=8, FF=4096, TOP_K=2.
"""

import os
import sys
import types

import numpy as np

for _p in ("/opt/trn_rl_repo",):
    if _p not in sys.path and os.path.isdir(_p):
        sys.path.insert(0, _p)

import ml_dtypes  # noqa: E402
import concourse.bass as bass  # noqa: E402
import concourse.mybir as mybir  # noqa: E402
import concourse.tile as tile  # noqa: E402
from concourse.bass_utils import run_bass_kernel_spmd  # noqa: E402


# ----------------------------------------------------------------------------
# NTFF profile hook shim: the container's stub `antenv` package lacks
# `axon_hooks`, so run_bass_kernel_spmd(trace=True) would silently skip
# profiling. Register the ctypes-based hook from trn_agent_boot ourselves.
def _install_ntff_hook():
    try:
        import antenv

        if "antenv.axon_hooks" in sys.modules:
            return
        m = types.ModuleType("antenv.axon_hooks")
        m._hook = None
        m.set_axon_ntff_profile_hook = lambda h: setattr(m, "_hook", h)
        m.get_axon_ntff_profile_hook = lambda: m._hook
        sys.modules["antenv.axon_hooks"] = m
        antenv.axon_hooks = m
        from trn_agent_boot.trn_boot import _ntff_profile_via_ctypes

        m.set_axon_ntff_profile_hook(
            _ntff_profile_via_ctypes("/opt/axon/libaxon_pjrt.so")
        )
    except Exception:
        pass


_install_ntff_hook()


# ----------------------------------------------------------------------------
# Post-pass: this container's walrus build rejects instructions carrying more
# than one sync wait ("Too many sync wait commands"). Move excess waits onto
# standalone NoOps inserted before the instruction on the same engine.
_WAITFIX_CTR = [0]


def _split_excess_waits(nc, max_waits: int = 1) -> int:
    n_split = 0
    for fn in nc.m.functions:
        for bb in fn.blocks:
            il = bb.instructions
            new = []
            changed = False
            for inst in il:
                si = inst.sync_info
                if si is not None and si.on_wait and len(si.on_wait) > max_waits:
                    waits = list(si.on_wait)
                    extra, keep = waits[:-max_waits], waits[-max_waits:]
                    for w in extra:
                        _WAITFIX_CTR[0] += 1
                        nop = mybir.InstNoOp(
                            name=f"wait-split-{_WAITFIX_CTR[0]}", ins=[], outs=[]
                        )
                        nop.engine = inst.engine
                        nop.sync_info = mybir.SyncInfo(on_wait=[w], on_update=[])
                        new.append(nop)
                        n_split += 1
                    inst.sync_info = mybir.SyncInfo(
                        on_wait=keep, on_update=list(si.on_update)
                    )
                    changed = True
                new.append(inst)
            if changed:
                il[:] = new
    return n_split


# ----------------------------------------------------------------------------
B, N, D, E, FF, TOP_K = 4, 1024, 1024, 8, 4096, 2
T = B * N
P = 128
KD = D // P  # 8 contraction chunks over D
KF = FF // P  # 32 contraction chunks over FF
NCORES = 8

F32 = mybir.dt.float32
AF = mybir.ActivationFunctionType

LAST_RESULT = None  # BassKernelResults of the most recent device dispatch


def _token_chunks(C):
    """Split [0, C) into near-equal chunks <= 512, multiples of 32.

    Equal-size chunks keep every matmul's free dim large (LDWEIGHTS stays
    hidden behind the stream) instead of 512+...+small-remainder."""
    n = -(-C // 512)
    base = (C // n) // 32 * 32
    rem = (C - base * n) // 32  # number of chunks that get +32
    sizes = [base + 32 if i < rem else base for i in range(n)]
    chunks = []
    off = 0
    for s in sizes:
        chunks.append((off, s))
        off += s
    assert off == C
    return chunks


def _build_program(C: int, dt, with_bias: bool):
    """One expert's fused SwiGLU MLP over C (padded) tokens.

    DRAM parameter layouts (host pre-arranges):
      xT : [KD, P, C]    xT[k, p, t] = x_gathered[t, k*128+p]
      w1 : [KF, P, KD*P] w1[f, p, k*128+c] = W1[k*128+p, f*128+c]
      wg : same as w1
      w2 : [KF, P, D]    w2[f, p, :] = W2[f*128+p, :]
      wc : [P, NTB]      wc[p, t] = combine_weight[t*128+p]
      b1f/bgf : [P, KF]  b1f[p, f] = b1[f*128+p]
      y  : [NTB, P, D]   y[t, p, :] = out[t*128+p, :]
    """
    NTB = -(-C // P)
    chunks = _token_chunks(C)
    # f=0 chunking: small leading chunk to shrink the kernel-head DMA wait
    if C >= 640:
        chunks0 = [(0, P)] + [(P + t0, tl) for (t0, tl) in _token_chunks(C - P)]
    else:
        chunks0 = chunks
    nc = bass.Bass()

    xT_d = nc.declare_dram_parameter("xT", [KD, P, C], dt, isOutput=False)
    w1_d = nc.declare_dram_parameter("w1", [KF, P, KD * P], dt, isOutput=False)
    wg_d = nc.declare_dram_parameter("wg", [KF, P, KD * P], dt, isOutput=False)
    w2_d = nc.declare_dram_parameter("w2", [KF, P, D], dt, isOutput=False)
    wc_d = nc.declare_dram_parameter("wc", [P, NTB], F32, isOutput=False)
    if with_bias:
        b1_d = nc.declare_dram_parameter("b1f", [P, KF], F32, isOutput=False)
        bg_d = nc.declare_dram_parameter("bgf", [P, KF], F32, isOutput=False)
    y_d = nc.declare_dram_parameter("y", [NTB, P, D], F32, isOutput=True)

    # Per-partition SBUF bytes if w2 is cached resident: xT + aT + w2 + streams
    sz = mybir.dt.size(dt)
    per_part = (KD * C + KF * C + KF * D) * sz + 31 * 1024
    cache_w2 = per_part <= 190 * 1024

    with tile.TileContext(nc) as tc:
        with tc.tile_pool(name="const", bufs=1) as constp:
            # Resident inputs go through the software DGE (gpsimd/Pool engine):
            # a separate DMA ring from the HWDGE one, so the w2/x preloads
            # don't queue behind (or throttle) the phase-A weight stream.
            xTt = constp.tile([P, KD, C], dt)
            # chunk-major emission: the first token chunk's pieces (needed by
            # the first accumulation group) lead the SWDGE ring
            for (t0, tl) in chunks0:
                for k in range(KD):
                    nc.gpsimd.dma_start(
                        out=xTt[:, k, t0 : t0 + tl], in_=xT_d[k, :, t0 : t0 + tl]
                    )
            wct = constp.tile([P, NTB], F32)
            nc.gpsimd.dma_start(out=wct[:], in_=wc_d[:])
            if with_bias:
                b1t = constp.tile([P, KF], F32)
                nc.gpsimd.dma_start(out=b1t[:], in_=b1_d[:])
                bgt = constp.tile([P, KF], F32)
                nc.gpsimd.dma_start(out=bgt[:], in_=bg_d[:])
            aT = constp.tile([P, KF, C], dt)
            w2t = constp.tile([P, KF, D], dt, name="w2t") if cache_w2 else None
            if cache_w2:
                for f in range(KF):
                    nc.gpsimd.dma_start(out=w2t[:, f, :], in_=w2_d[f])

            # ---- Phase A: h^T/g^T per 128-wide FF chunk; a = silu(g) * h ----
            with tc.tile_pool(name="wstream", bufs=4) as wp, tc.tile_pool(
                name="hgsb", bufs=4
            ) as hgp, tc.tile_pool(name="psA", bufs=3, space="PSUM") as psA:
                for f in range(KF):
                    w1f = wp.tile([P, KD, P], dt, tag="w1f")
                    wgf = wp.tile([P, KD, P], dt, tag="wgf")
                    if f == 0:
                        # first matmul's critical path: halves land on two queues
                        hk = KD // 2
                        nc.sync.dma_start(out=w1f[:, :hk, :], in_=w1_d[0, :, : hk * P])
                        nc.sync.dma_start(out=w1f[:, hk:, :], in_=w1_d[0, :, hk * P :])
                        nc.sync.dma_start(out=wgf[:, :hk, :], in_=wg_d[0, :, : hk * P])
                        nc.sync.dma_start(out=wgf[:, hk:, :], in_=wg_d[0, :, hk * P :])
                    else:
                        nc.sync.dma_start(out=w1f[:], in_=w1_d[f])
                        nc.sync.dma_start(out=wgf[:], in_=wg_d[f])
                    # f == 0 starts with a small 128-token chunk so the first
                    # matmuls wait on ~300KB of DMA instead of ~1.3MB
                    f_chunks = chunks0 if f == 0 else chunks
                    for (t0, tl) in f_chunks:
                        hps = psA.tile([P, 512], F32, tag="h")
                        gps = psA.tile([P, 512], F32, tag="g")
                        for k in range(KD):
                            nc.tensor.matmul(
                                hps[:, :tl],
                                w1f[:, k, :],
                                xTt[:, k, t0 : t0 + tl],
                                start=(k == 0),
                                stop=(k == KD - 1),
                            )
                        for k in range(KD):
                            nc.tensor.matmul(
                                gps[:, :tl],
                                wgf[:, k, :],
                                xTt[:, k, t0 : t0 + tl],
                                start=(k == 0),
                                stop=(k == KD - 1),
                            )
                        sg = hgp.tile([P, 512], F32, tag="sg")
                        if with_bias:
                            nc.scalar.activation(
                                sg[:, :tl], gps[:, :tl], AF.Silu, bias=bgt[:, f : f + 1]
                            )
                            hb = hgp.tile([P, 512], F32, tag="hb")
                            nc.scalar.activation(
                                hb[:, :tl],
                                hps[:, :tl],
                                AF.Identity,
                                bias=b1t[:, f : f + 1],
                            )
                            nc.vector.tensor_mul(
                                aT[:, f, t0 : t0 + tl], sg[:, :tl], hb[:, :tl]
                            )
                        else:
                            nc.scalar.activation(sg[:, :tl], gps[:, :tl], AF.Silu)
                            nc.vector.tensor_mul(
                                aT[:, f, t0 : t0 + tl], sg[:, :tl], hps[:, :tl]
                            )

            # ---- Phase B: y[t, :] = (a^T)^T @ W2, scaled by combine weight ----
            with tc.tile_pool(name="w2p", bufs=3) as w2p, tc.tile_pool(
                name="ysb", bufs=4
            ) as yp, tc.tile_pool(name="psB", bufs=1, space="PSUM") as psB:
                GROUP = 4  # 4 token chunks x 2 halves of D = 8 PSUM banks
                starts = list(range(0, NTB, GROUP))
                groups = [
                    list(range(g0, min(g0 + GROUP, NTB))) for g0 in starts
                ]
                if len(groups[-1]) > 1:
                    # end on a 1-chunk group: the after-last-matmul eviction
                    # tail is 2 tiles instead of 8
                    groups = groups[:-1] + [groups[-1][:-1], groups[-1][-1:]]
                for ts in groups:
                    g0 = ts[0]
                    ytiles = {}
                    for t in ts:
                        for dn in range(2):
                            ytiles[(t, dn)] = psB.tile(
                                [P, 512],
                                F32,
                                tag=f"y{t - g0}_{dn}",
                                name=f"ypsum_{t - g0}_{dn}",
                            )
                    if cache_w2:
                        # K-contiguous: all 32 f-accumulations back-to-back
                        # into one PSUM bank before moving to the next — the
                        # first chains run while phase A's tail still drains
                        # its banks, and the PE never cycles banks mid-chain.
                        for t in ts:
                            mt = min(P, C - t * P)
                            for dn in range(2):
                                for f in range(KF):
                                    nc.tensor.matmul(
                                        ytiles[(t, dn)][:mt],
                                        aT[:, f, t * P : t * P + mt],
                                        w2t[:, f, dn * 512 : (dn + 1) * 512],
                                        start=(f == 0),
                                        stop=(f == KF - 1),
                                    )
                    else:
                        for f in range(KF):
                            w2f = w2p.tile([P, D], dt, tag="w2f")
                            nc.sync.dma_start(out=w2f[:], in_=w2_d[f])
                            for t in ts:
                                mt = min(P, C - t * P)
                                for dn in range(2):
                                    nc.tensor.matmul(
                                        ytiles[(t, dn)][:mt],
                                        aT[:, f, t * P : t * P + mt],
                                        w2f[:, dn * 512 : (dn + 1) * 512],
                                        start=(f == 0),
                                        stop=(f == KF - 1),
                                    )
                    for t in ts:
                        mt = min(P, C - t * P)
                        for dn in range(2):
                            ysb = yp.tile([P, 512], F32, tag="ysb")
                            if (t + dn) % 2 == 0:
                                nc.scalar.activation(
                                    ysb[:mt],
                                    ytiles[(t, dn)][:mt],
                                    AF.Copy,
                                    scale=wct[:mt, t : t + 1],
                                )
                            else:
                                nc.vector.tensor_scalar_mul(
                                    ysb[:mt], ytiles[(t, dn)][:mt], wct[:mt, t : t + 1]
                                )
                            # halves on two queues: the final tile's store —
                            # which the end-of-kernel drain waits on — takes
                            # ~4us instead of ~8us through a single queue
                            nc.sync.dma_start(
                                out=y_d[t, :mt, dn * 512 : dn * 512 + 256],
                                in_=ysb[:mt, :256],
                            )
                            nc.sync.dma_start(
                                out=y_d[t, :mt, dn * 512 + 256 : (dn + 1) * 512],
                                in_=ysb[:mt, 256:],
                            )

    _split_excess_waits(nc)
    return nc


_PROG_CACHE = {}


def _get_program(C, dt_name, with_bias):
    key = (C, dt_name, with_bias)
    if key not in _PROG_CACHE:
        dt = {"bf16": mybir.dt.bfloat16, "f32r": mybir.dt.float32r, "f32": F32}[
            dt_name
        ]
        _PROG_CACHE[key] = _build_program(C, dt, with_bias)
    return _PROG_CACHE[key]


# ----------------------------------------------------------------------------
def _route(xf, Wr):
    """Replicate the reference router bit-for-bit (jax on CPU), falling back
    to float64 numpy for the picks if jax is unavailable.

    Returns probs [T,E] f32, topi [T,K] int, topv_norm [T,K] f32."""
    try:
        import jax
        import jax.numpy as jnp

        cpu = jax.devices("cpu")[0]
        with jax.default_device(cpu):
            logits = jnp.asarray(xf) @ jnp.asarray(Wr).T
            probs = jax.nn.softmax(logits, axis=-1)
            topv, topi = jax.lax.top_k(probs, TOP_K)
            probs = np.asarray(probs)
            topv = np.asarray(topv)
            topi = np.asarray(topi)
    except Exception:
        l64 = xf.astype(np.float64) @ Wr.astype(np.float64).T
        logits = (xf @ Wr.T).astype(np.float32)
        m = logits.max(-1, keepdims=True)
        e = np.exp(logits - m)
        probs = e / e.sum(-1, keepdims=True)
        topi = np.argsort(-l64, axis=-1, kind="stable")[:, :TOP_K]
        topv = np.take_along_axis(probs, topi, -1)
    topv = topv / (topv.sum(-1, keepdims=True) + 1e-6)
    return probs.astype(np.float32), topi, topv.astype(np.float32)


def _prep_weights(W1, Wg, W2, np_dt):
    """Host re-layouts (cached on id) so device DMAs are partition-contiguous."""
    key = (id(W1), id(Wg), id(W2), np_dt)
    cached = _prep_weights._cache.get(key)
    if cached is not None:
        return cached
    # w1[e][f, p, k*128+c] = W1[e, k*128+p, f*128+c]
    w1 = np.ascontiguousarray(
        W1.reshape(E, KD, P, KF, P).transpose(0, 3, 2, 1, 4).reshape(E, KF, P, KD * P)
    ).astype(np_dt)
    wg = np.ascontiguousarray(
        Wg.reshape(E, KD, P, KF, P).transpose(0, 3, 2, 1, 4).reshape(E, KF, P, KD * P)
    ).astype(np_dt)
    w2 = np.ascontiguousarray(W2.reshape(E, KF, P, D)).astype(np_dt)
    _prep_weights._cache = {key: (w1, wg, w2)}
    return w1, wg, w2


_prep_weights._cache = {}


def _expert_mlp_host(xg, W1e, b1e, Wge, bge, W2e, b2e):
    """fp32 numpy expert forward for the (pathological) host fallback."""
    h = xg @ W1e + b1e
    g = xg @ Wge + bge
    sg = g / (1.0 + np.exp(-g))
    return (sg * h) @ W2e + b2e


def kernel(x, Wr, W1, b1, Wg, bg, W2, b2):
    global LAST_RESULT
    dt_name = "bf16"
    np_dt = ml_dtypes.bfloat16

    x = np.asarray(x, np.float32)
    Wr = np.asarray(Wr, np.float32)
    W1 = np.asarray(W1, np.float32)
    Wg = np.asarray(Wg, np.float32)
    W2 = np.asarray(W2, np.float32)
    b1 = np.asarray(b1, np.float32)
    bg = np.asarray(bg, np.float32)
    b2 = np.asarray(b2, np.float32)

    xf = x.reshape(T, D)
    probs, topi, topv = _route(xf, Wr)

    # combine weights comb[t, e]
    comb = np.zeros((T, E), np.float32)
    np.add.at(comb, (np.arange(T)[:, None], topi), topv)

    # tokens per expert (ascending token order, matching reference accumulation);
    # membership from topi, not comb != 0, so a 0-weight selection still routes
    sel = np.zeros((T, E), bool)
    sel[np.arange(T)[:, None], topi] = True
    idx = [np.nonzero(sel[:, e])[0] for e in range(E)]
    counts = np.array([len(i) for i in idx])

    # Device capacity: padding costs PE time per token (phase A) and per
    # 128-token PSUM chunk (phase B), so cap at a clean 1024 and compute the
    # few overflow (token, expert) pairs on the host in exact fp32.
    C = max(512, int(np.ceil(counts.max() / 32)) * 32)
    if counts.max() > 1024:
        C = 1024
    NTB = -(-C // P)

    if C > 1792:
        # Pathologically imbalanced routing: capacity would overflow SBUF.
        # Never triggers for a near-uniform router; correctness fallback only.
        out = np.zeros((T, D), np.float32)
        for e in range(E):
            ic = idx[e]
            if len(ic):
                out[ic] += comb[ic, e][:, None] * _expert_mlp_host(
                    xf[ic], W1[e], b1[e], Wg[e], bg[e], W2[e], b2[e]
                )
        countsf = np.zeros((E,), np.float32)
        np.add.at(countsf, topi.reshape(-1), np.float32(1.0))
        frac = countsf / (countsf.sum() + np.float32(1e-6))
        lb = np.float32((frac * probs.sum(0)).sum() * E)
        return out.reshape(B, N, D), lb

    with_bias = bool(b1.any() or bg.any())
    nc = _get_program(C, dt_name, with_bias)

    w1h, wgh, w2h = _prep_weights(W1, Wg, W2, np_dt)

    in_maps = []
    for c in range(NCORES):
        ic = idx[c][:C]
        ncnt = len(ic)
        xg = np.zeros((C, D), np.float32)
        xg[:ncnt] = xf[ic]
        xT = np.ascontiguousarray(xg.T.reshape(KD, P, C)).astype(np_dt)
        wc = np.zeros((NTB * P,), np.float32)
        wc[:ncnt] = comb[ic, c]
        m = {
            "xT": xT,
            "w1": w1h[c],
            "wg": wgh[c],
            "w2": w2h[c],
            "wc": np.ascontiguousarray(wc.reshape(NTB, P).T),
        }
        if with_bias:
            m["b1f"] = np.ascontiguousarray(b1[c].reshape(KF, P).T)
            m["bgf"] = np.ascontiguousarray(bg[c].reshape(KF, P).T)
        in_maps.append(m)

    res = run_bass_kernel_spmd(nc, in_maps, list(range(NCORES)))
    LAST_RESULT = res

    out = np.zeros((T, D), np.float32)
    for c in range(NCORES):
        ic = idx[c][:C]
        yc = np.asarray(res.results[c]["y"], np.float32).reshape(NTB * P, D)
        out[ic] += yc[: len(ic)]
        ov = idx[c][C:]
        if len(ov):
            # b2 excluded here: the global `comb @ b2` term below already
            # covers every selected (token, expert) pair
            out[ov] += comb[ov, c][:, None] * _expert_mlp_host(
                xf[ov], W1[c], b1[c], Wg[c], bg[c], W2[c], np.float32(0.0)
            )

    if b2.any():
        out += comb @ b2

    # load-balance aux loss, replicated from the reference
    countsf = np.zeros((E,), np.float32)
    np.add.at(countsf, topi.reshape(-1), np.float32(1.0))
    frac = countsf / (countsf.sum() + np.float32(1e-6))
    lb = np.float32((frac * probs.sum(0)).sum() * E)

    return out.reshape(B, N, D), lb
